# revision 16
# baseline (speedup 1.0000x reference)
"""Trainium2 Bass kernel for nn_Attention_8675833938565.

Computes, per batch element b (one NeuronCore each, 8 cores):
    attn = softmax(mask ? -inf : output @ context^T)          [Q, K]
    mix  = attn @ context                                     [Q, H]
    out  = tanh(concat([mix, output]) @ W_out^T + b_out)      [Q, H]
returns (out, attn) stacked over the 8 batch elements.

Layout strategy per core (Q=1024, K=4096, H=512, fp32):
  - context^T, output^T, W_out^T are built on-chip with identity-matmul
    transposes (PE), kept resident in SBUF.
  - scores S[qtile] accumulate in PSUM over 4 h-steps; eviction fuses
    mask-add (-1e30 * mask) + running row-max in one DVE op.
  - ACT does exp(x - rowmax) with fused row-sum; DVE reciprocal; GPSIMD
    normalizes in place; result DMAs out and is PE-transposed (bf16) to
    feed the mix matmul against a resident bf16 copy of context.
  - proj matmul consumes mix^T / output^T tiles with W_out^T, accumulates
    b_out via a rank-1 ones-matmul, ACT tanh-evicts, DMA out.
"""

import sys

if "/opt/trn_rl_repo" not in sys.path:
    sys.path.insert(0, "/opt/trn_rl_repo")

from contextlib import ExitStack

import numpy as np

import concourse.bass as bass
import concourse.mybir as mybir
import concourse.tile as tile
from concourse.masks import make_identity

B, Q, K, H = 8, 1024, 4096, 512
P = 128
QT, KT, HT = Q // P, K // P, H // P  # 8, 32, 4
NCH = K // 512  # 8 score chunks per q-tile
F32 = mybir.dt.float32
BF16 = mybir.dt.bfloat16
F32R = mybir.dt.float32r
U8 = mybir.dt.uint8
NEGBIG = -1.0e30


_TC = tile.TileContext


def _split_multi_waits(nc):
    """The walrus build in this container accepts at most ONE sem wait per
    instruction; Tile attaches several. Hoist extra waits onto same-engine
    NoOps inserted immediately before the over-subscribed instruction."""
    counter = 0
    for f in nc.m.functions:
        for blk in f.blocks:
            il = blk.instructions
            new_il = []
            changed = False
            for ins in il:
                si = ins.sync_info
                if si is not None and si.on_wait and len(si.on_wait) > 1:
                    waits = list(si.on_wait)
                    for w in waits[:-1]:
                        counter += 1
                        nop = mybir.InstNoOp(
                            name=f"I-waitsplit-{counter}", ins=[], outs=[]
                        )
                        nop.engine = ins.engine
                        nop.sync_info = mybir.SyncInfo(on_wait=[w], on_update=[])
                        new_il.append(nop)
                    si.on_wait = waits[-1:]
                    ins.sync_info = si
                    changed = True
                new_il.append(ins)
            if changed:
                blk.instructions = new_il


def _setup_jax_cache():
    try:
        import jax

        jax.config.update("jax_compilation_cache_dir", "/tmp/jax_neff_cache")
        jax.config.update("jax_persistent_cache_min_entry_size_bytes", -1)
        jax.config.update("jax_persistent_cache_min_compile_time_secs", 0.0)
    except Exception:
        pass


def build(loop=1):
    nc = bass.Bass()
    x_out = nc.declare_dram_parameter("x_output", [Q, H], F32, isOutput=False)
    x_ctx = nc.declare_dram_parameter("x_context", [K, H], F32, isOutput=False)
    x_mask = nc.declare_dram_parameter("x_mask", [Q, K], U8, isOutput=False)
    x_w = nc.declare_dram_parameter("x_wout", [H, 2 * H], F32, isOutput=False)
    x_b = nc.declare_dram_parameter("x_bout", [1, H], F32, isOutput=False)
    y_out = nc.declare_dram_parameter("y_out", [Q, H], F32, isOutput=True)
    y_attn = nc.declare_dram_parameter("y_attn", [Q, K], F32, isOutput=True)

    AL = mybir.AluOpType
    AF = mybir.ActivationFunctionType
    AX = mybir.AxisListType

    with _TC(nc) as tc, ExitStack() as ctx:
        if loop > 1:
            ctx.enter_context(tc.For_i(0, loop, 1))
        const = ctx.enter_context(tc.tile_pool(name="const", bufs=1))
        ident = const.tile([P, P], F32)
        make_identity(nc, ident[:])
        ones_stage = const.tile([1, P], F32)
        nc.gpsimd.memset(ones_stage[:], 1.0)
        ones = const.tile([1, P], F32R)
        nc.vector.tensor_copy(ones[:], ones_stage[:])
        bias_stage = const.tile([1, H], F32)
        nc.sync.dma_start(out=bias_stage[:], in_=x_b[:])
        bias_sb = const.tile([1, H], F32R)
        nc.vector.tensor_copy(bias_sb[:], bias_stage[:])

        pers = ctx.enter_context(tc.tile_pool(name="pers", bufs=1))
        ctxT = [pers.tile([P, K], F32R, name=f"ctxT{h}", tag=f"ctxT{h}") for h in range(HT)]
        outT = [pers.tile([P, Q], F32R, name=f"outT{h}", tag=f"outT{h}") for h in range(HT)]
        wT = [pers.tile([P, H], F32R, name=f"wT{c}", tag=f"wT{c}") for c in range(2 * H // P)]
        cbf = [pers.tile([P, H], BF16, name=f"cbf{k}", tag=f"cbf{k}") for k in range(KT)]

        # ---------------- prep: build outT / ctxT / cbf / wT ----------------
        with tc.tile_pool(name="stage", bufs=6) as stage, tc.tile_pool(
            name="wstage", bufs=4
        ) as wstage, tc.tile_pool(name="prep_ps", bufs=3, space="PSUM") as pp:
            # output^T
            for qb in range(QT // 4):
                sts = []
                for j in range(4):
                    qt = qb * 4 + j
                    st = stage.tile([P, H], F32, tag="stage")
                    nc.sync.dma_start(out=st[:], in_=x_out[qt * P : (qt + 1) * P, :])
                    sts.append(st)
                for h in range(HT):
                    ps = pp.tile([P, 512], F32)
                    for j in range(4):
                        nc.tensor.matmul(
                            ps[:, j * P : (j + 1) * P],
                            lhsT=sts[j][:, h * P : (h + 1) * P],
                            rhs=ident[:],
                            is_transpose=True,
                            start=True,
                            stop=True,
                        )
                    nc.vector.tensor_copy(
                        outT[h][:, qb * 512 : (qb + 1) * 512], ps[:]
                    )
            # context^T (+ bf16 copy of context)
            for kb in range(KT // 4):
                sts = []
                for j in range(4):
                    kt = kb * 4 + j
                    st = stage.tile([P, H], F32, tag="stage")
                    nc.sync.dma_start(out=st[:], in_=x_ctx[kt * P : (kt + 1) * P, :])
                    nc.vector.tensor_copy(cbf[kt][:], st[:])
                    sts.append(st)
                for h in range(HT):
                    ps = pp.tile([P, 512], F32)
                    for j in range(4):
                        nc.tensor.matmul(
                            ps[:, j * P : (j + 1) * P],
                            lhsT=sts[j][:, h * P : (h + 1) * P],
                            rhs=ident[:],
                            is_transpose=True,
                            start=True,
                            stop=True,
                        )
                    nc.scalar.copy(ctxT[h][:, kb * 512 : (kb + 1) * 512], ps[:])
            # W_out^T
            wst = []
            for r in range(4):
                st = wstage.tile([P, 2 * H], F32, tag="wstage")
                nc.sync.dma_start(out=st[:], in_=x_w[r * P : (r + 1) * P, :])
                wst.append(st)
            for c in range(2 * H // P):
                ps = pp.tile([P, 512], F32)
                for r in range(4):
                    nc.tensor.matmul(
                        ps[:, r * P : (r + 1) * P],
                        lhsT=wst[r][:, c * P : (c + 1) * P],
                        rhs=ident[:],
                        is_transpose=True,
                        start=True,
                        stop=True,
                    )
                nc.vector.tensor_copy(wT[c][:], ps[:])

        # ---------------- main loop over q-tiles ----------------
        mq = ctx.enter_context(tc.tile_pool(name="mask", bufs=2))
        mnp = ctx.enter_context(tc.tile_pool(name="maskneg", bufs=2))
        ap = ctx.enter_context(tc.tile_pool(name="attn", bufs=2))
        atp = ctx.enter_context(tc.tile_pool(name="attnT", bufs=4))
        stp = ctx.enter_context(tc.tile_pool(name="stats", bufs=4))
        mxp = ctx.enter_context(tc.tile_pool(name="mix", bufs=2))
        mtp = ctx.enter_context(tc.tile_pool(name="mixT", bufs=2))
        osb = ctx.enter_context(tc.tile_pool(name="osb", bufs=2))
        sc_ps = ctx.enter_context(tc.tile_pool(name="sc_ps", bufs=4, space="PSUM"))
        tr_ps = ctx.enter_context(tc.tile_pool(name="tr_ps", bufs=2, space="PSUM"))
        mix_ps = ctx.enter_context(tc.tile_pool(name="mix_ps", bufs=1, space="PSUM"))
        pr_ps = ctx.enter_context(tc.tile_pool(name="pr_ps", bufs=1, space="PSUM"))

        for qt in range(QT):
            qsl = slice(qt * P, (qt + 1) * P)
            mk = mq.tile([P, K], U8)
            nc.sync.dma_start(out=mk[:], in_=x_mask[qsl, :])
            mneg = mnp.tile([P, K], BF16)
            nc.gpsimd.tensor_scalar_mul(mneg[:], mk[:], NEGBIG)

            att = ap.tile([P, K], F32)
            stats = stp.tile([P, 16], F32)

            # scores: 4 quarters x 2 chunks; PSUM accumulates over h
            for quarter in range(4):
                pss = [
                    sc_ps.tile([P, 512], F32, name=f"scps{quarter}_{i2}", tag="scps")
                    for i2 in range(2)
                ]
                for h in range(HT):
                    for i in range(2):
                        n = quarter * 2 + i
                        nc.tensor.matmul(
                            pss[i][:],
                            lhsT=outT[h][:, qsl],
                            rhs=ctxT[h][:, n * 512 : (n + 1) * 512],
                            start=(h == 0),
                            stop=(h == HT - 1),
                        )
                for i in range(2):
                    n = quarter * 2 + i
                    nc.vector.tensor_tensor(
                        out=att[:, n * 512 : (n + 1) * 512],
                        in0=pss[i][:],
                        in1=mneg[:, n * 512 : (n + 1) * 512],
                        op=AL.add,
                    )
                    nc.vector.tensor_reduce(
                        out=stats[:, n : n + 1],
                        in_=att[:, n * 512 : (n + 1) * 512],
                        axis=AX.X,
                        op=AL.max,
                    )

            # softmax over the full row
            nc.vector.tensor_reduce(
                out=stats[:, 8:9], in_=stats[:, 0:NCH], axis=AX.X, op=AL.max,
                negate=True,
            )
            nc.scalar.activation(
                out=att[:], in_=att[:], func=AF.Exp,
                bias=stats[:, 8:9], scale=1.0, accum_out=stats[:, 9:10],
            )
            nc.vector.reciprocal(stats[:, 10:11], stats[:, 9:10])
            nc.gpsimd.tensor_scalar_mul(att[:], att[:], stats[:, 10:11])

            nc.sync.dma_start(out=y_attn[qsl, :], in_=att[:])

            # attn^T (bf16) + mix = attn @ context, software-pipelined on PE
            mxps = mix_ps.tile([P, 512], F32)
            prev = None
            for kb in range(KT // 4):
                tp = tr_ps.tile([P, 512], F32)
                for j in range(4):
                    nc.tensor.matmul(
                        tp[:, j * P : (j + 1) * P],
                        lhsT=att[:, (kb * 4 + j) * P : (kb * 4 + j + 1) * P],
                        rhs=ident[:],
                        is_transpose=True,
                        start=True,
                        stop=True,
                    )
                at_sb = atp.tile([P, 512], BF16)
                nc.scalar.copy(at_sb[:], tp[:])
                if prev is not None:
                    pkb, pat = prev
                    for j in range(4):
                        kt = pkb * 4 + j
                        nc.tensor.matmul(
                            mxps[:],
                            lhsT=pat[:, j * P : (j + 1) * P],
                            rhs=cbf[kt][:],
                            start=(kt == 0),
                            stop=False,
                        )
                prev = (kb, at_sb)
            pkb, pat = prev
            for j in range(4):
                kt = pkb * 4 + j
                nc.tensor.matmul(
                    mxps[:],
                    lhsT=pat[:, j * P : (j + 1) * P],
                    rhs=cbf[kt][:],
                    start=False,
                    stop=(kt == KT - 1),
                )
            mx = mxp.tile([P, H], F32)
            nc.scalar.copy(mx[:], mxps[:])

            # mix^T
            mtps = tr_ps.tile([P, 512], F32, name="mtps", tag="tp")
            for j in range(HT):
                nc.tensor.matmul(
                    mtps[:, j * P : (j + 1) * P],
                    lhsT=mx[:, j * P : (j + 1) * P],
                    rhs=ident[:],
                    is_transpose=True,
                    start=True,
                    stop=True,
                )
            mt = mtp.tile([P, H], F32R)
            nc.vector.tensor_copy(mt[:], mtps[:])

            # proj: out = tanh([mix, output] @ W_out^T + b_out)
            prps = pr_ps.tile([P, H], F32)
            for c in range(2 * H // P):
                if c < HT:
                    lhsT = mt[:, c * P : (c + 1) * P]
                else:
                    lhsT = outT[c - HT][:, qsl]
                nc.tensor.matmul(
                    prps[:],
                    lhsT=lhsT,
                    rhs=wT[c][:],
                    start=(c == 0),
                    stop=False,
                )
            nc.tensor.matmul(
                prps[:],
                lhsT=ones[:, :],
                rhs=bias_sb[:],
                start=False,
                stop=True,
            )

            ot = osb.tile([P, H], F32)
            nc.scalar.activation(out=ot[:], in_=prps[:], func=AF.Tanh)
            nc.sync.dma_start(out=y_out[qsl, :], in_=ot[:])

    return nc


_NC_CACHE = None


def _get_nc():
    global _NC_CACHE
    if _NC_CACHE is None:
        _NC_CACHE = build()
    return _NC_CACHE


def kernel(output, context, mask, W_out, b_out):
    from concourse.bass_utils import run_bass_kernel_spmd

    _setup_jax_cache()

    output = np.ascontiguousarray(np.asarray(output), dtype=np.float32)
    context = np.ascontiguousarray(np.asarray(context), dtype=np.float32)
    mask_u8 = np.ascontiguousarray(np.asarray(mask)).astype(np.uint8)
    W = np.ascontiguousarray(np.asarray(W_out), dtype=np.float32)
    b = np.ascontiguousarray(np.asarray(b_out), dtype=np.float32).reshape(1, H)

    nc = _get_nc()
    _split_multi_waits(nc)
    in_maps = [
        {
            "x_output": output[i],
            "x_context": context[i],
            "x_mask": mask_u8[i],
            "x_wout": W,
            "x_bout": b,
        }
        for i in range(B)
    ]
    res = run_bass_kernel_spmd(nc, in_maps, list(range(B)))
    out = np.stack([res.results[i]["y_out"] for i in range(B)])
    attn = np.stack([res.results[i]["y_attn"] for i in range(B)])
    return out, attn


# revision 20
# speedup vs baseline: 4.2969x; 4.2969x over previous
"""Trainium2 Bass kernel for nn_Attention_8675833938565.

Computes, per batch element b (one NeuronCore each, 8 cores):
    attn = softmax(mask ? -inf : output @ context^T)          [Q, K]
    mix  = attn @ context                                     [Q, H]
    out  = tanh(concat([mix, output]) @ W_out^T + b_out)      [Q, H]
returns (out, attn) stacked over the 8 batch elements.

Layout strategy per core (Q=1024, K=4096, H=512, fp32):
  - context^T, output^T, W_out^T are built on-chip with identity-matmul
    transposes (PE), kept resident in SBUF.
  - scores S[qtile] accumulate in PSUM over 4 h-steps; eviction fuses
    mask-add (-1e30 * mask) + running row-max in one DVE op.
  - ACT does exp(x - rowmax) with fused row-sum; DVE reciprocal; GPSIMD
    normalizes in place; result DMAs out and is PE-transposed (bf16) to
    feed the mix matmul against a resident bf16 copy of context.
  - proj matmul consumes mix^T / output^T tiles with W_out^T, accumulates
    b_out via a rank-1 ones-matmul, ACT tanh-evicts, DMA out.
"""

import sys

if "/opt/trn_rl_repo" not in sys.path:
    sys.path.insert(0, "/opt/trn_rl_repo")

from contextlib import ExitStack

import numpy as np

import concourse.bass as bass
import concourse.mybir as mybir
import concourse.tile as tile
from concourse.masks import make_identity

B, Q, K, H = 8, 1024, 4096, 512
P = 128
QT, KT, HT = Q // P, K // P, H // P  # 8, 32, 4
NCH = K // 512  # 8 score chunks per q-tile
F32 = mybir.dt.float32
BF16 = mybir.dt.bfloat16
F32R = mybir.dt.float32r
F16 = mybir.dt.float16
U8 = mybir.dt.uint8
NEGBIG = -1.0e30


_TC = tile.TileContext


def _split_multi_waits(nc):
    """The walrus build in this container accepts at most ONE sem wait per
    instruction; Tile attaches several. Hoist extra waits onto same-engine
    NoOps inserted immediately before the over-subscribed instruction."""
    counter = 0
    for f in nc.m.functions:
        for blk in f.blocks:
            il = blk.instructions
            new_il = []
            changed = False
            for ins in il:
                si = ins.sync_info
                if si is not None and si.on_wait and len(si.on_wait) > 1:
                    waits = list(si.on_wait)
                    for w in waits[:-1]:
                        counter += 1
                        nop = mybir.InstNoOp(
                            name=f"I-waitsplit-{counter}", ins=[], outs=[]
                        )
                        nop.engine = ins.engine
                        nop.sync_info = mybir.SyncInfo(on_wait=[w], on_update=[])
                        new_il.append(nop)
                    si.on_wait = waits[-1:]
                    ins.sync_info = si
                    changed = True
                new_il.append(ins)
            if changed:
                blk.instructions = new_il


def _setup_jax_cache():
    try:
        import jax

        jax.config.update("jax_compilation_cache_dir", "/tmp/jax_neff_cache")
        jax.config.update("jax_persistent_cache_min_entry_size_bytes", -1)
        jax.config.update("jax_persistent_cache_min_compile_time_secs", 0.0)
    except Exception:
        pass


def build(loop=1):
    nc = bass.Bass()
    x_out = nc.declare_dram_parameter("x_output", [Q, H], F32, isOutput=False)
    x_ctx = nc.declare_dram_parameter("x_context", [K, H], F32, isOutput=False)
    x_mask = nc.declare_dram_parameter("x_mask", [Q, K], U8, isOutput=False)
    x_w = nc.declare_dram_parameter("x_wout", [H, 2 * H], F32, isOutput=False)
    x_b = nc.declare_dram_parameter("x_bout", [1, H], F32, isOutput=False)
    y_out = nc.declare_dram_parameter("y_out", [Q, H], F32, isOutput=True)
    y_attn = nc.declare_dram_parameter("y_attn", [Q, K], F32, isOutput=True)

    AL = mybir.AluOpType
    AF = mybir.ActivationFunctionType
    AX = mybir.AxisListType

    with _TC(nc) as tc, ExitStack() as ctx:
        if loop > 1:
            ctx.enter_context(tc.For_i(0, loop, 1))
        const = ctx.enter_context(tc.tile_pool(name="const", bufs=1))
        ident = const.tile([P, P], F32)
        make_identity(nc, ident[:])
        ones = const.tile([1, P], F16)
        nc.gpsimd.memset(ones[:], 1.0)
        bias_stage = const.tile([1, H], F32)
        nc.sync.dma_start(out=bias_stage[:], in_=x_b[:])
        bias_sb = const.tile([1, H], F16)
        nc.vector.tensor_copy(bias_sb[:], bias_stage[:])

        pers = ctx.enter_context(tc.tile_pool(name="pers", bufs=1))
        ctxTh = [pers.tile([P, K], F16, name=f"ctxTh{h}", tag=f"ctxTh{h}") for h in range(HT)]
        ctxTl = [pers.tile([P, K], F16, name=f"ctxTl{h}", tag=f"ctxTl{h}") for h in range(HT)]
        outTh = [pers.tile([P, Q], F16, name=f"outTh{h}", tag=f"outTh{h}") for h in range(HT)]
        outTl = [pers.tile([P, Q], F16, name=f"outTl{h}", tag=f"outTl{h}") for h in range(HT)]
        wT = [pers.tile([P, H], F16, name=f"wT{c}", tag=f"wT{c}") for c in range(2 * H // P)]
        cbf = [pers.tile([P, H], F16, name=f"cbf{k}", tag=f"cbf{k}") for k in range(KT)]

        # ---------------- prep: build outT / ctxT / cbf / wT ----------------
        with tc.tile_pool(name="stage", bufs=6) as stage, tc.tile_pool(
            name="wstage", bufs=4
        ) as wstage, tc.tile_pool(name="prep_ps", bufs=3, space="PSUM") as pp:
            # output^T
            for qb in range(QT // 4):
                sts = []
                for j in range(4):
                    qt = qb * 4 + j
                    st = stage.tile([P, H], F32, tag="stage")
                    nc.sync.dma_start(out=st[:], in_=x_out[qt * P : (qt + 1) * P, :])
                    sts.append(st)
                for h in range(HT):
                    ps = pp.tile([P, 512], F32)
                    for j in range(4):
                        nc.tensor.matmul(
                            ps[:, j * P : (j + 1) * P],
                            lhsT=sts[j][:, h * P : (h + 1) * P],
                            rhs=ident[:],
                            is_transpose=True,
                            start=True,
                            stop=True,
                        )
                    qbs = slice(qb * 512, (qb + 1) * 512)
                    nc.scalar.copy(outTh[h][:, qbs], ps[:])
                    nc.vector.tensor_tensor(
                        out=outTl[h][:, qbs], in0=ps[:], in1=outTh[h][:, qbs],
                        op=AL.subtract,
                    )
            # context^T (+ bf16 copy of context)
            for kb in range(KT // 4):
                sts = []
                for j in range(4):
                    kt = kb * 4 + j
                    st = stage.tile([P, H], F32, tag="stage")
                    nc.sync.dma_start(out=st[:], in_=x_ctx[kt * P : (kt + 1) * P, :])
                    nc.vector.tensor_copy(cbf[kt][:], st[:])
                    sts.append(st)
                for h in range(HT):
                    ps = pp.tile([P, 512], F32)
                    for j in range(4):
                        nc.tensor.matmul(
                            ps[:, j * P : (j + 1) * P],
                            lhsT=sts[j][:, h * P : (h + 1) * P],
                            rhs=ident[:],
                            is_transpose=True,
                            start=True,
                            stop=True,
                        )
                    kbs = slice(kb * 512, (kb + 1) * 512)
                    nc.scalar.copy(ctxTh[h][:, kbs], ps[:])
                    nc.vector.tensor_tensor(
                        out=ctxTl[h][:, kbs], in0=ps[:], in1=ctxTh[h][:, kbs],
                        op=AL.subtract,
                    )
            # W_out^T
            wst = []
            for r in range(4):
                st = wstage.tile([P, 2 * H], F32, tag="wstage")
                nc.sync.dma_start(out=st[:], in_=x_w[r * P : (r + 1) * P, :])
                wst.append(st)
            for c in range(2 * H // P):
                ps = pp.tile([P, 512], F32)
                for r in range(4):
                    nc.tensor.matmul(
                        ps[:, r * P : (r + 1) * P],
                        lhsT=wst[r][:, c * P : (c + 1) * P],
                        rhs=ident[:],
                        is_transpose=True,
                        start=True,
                        stop=True,
                    )
                nc.vector.tensor_copy(wT[c][:], ps[:])

        # ---------------- main loop over q-tiles ----------------
        mq = ctx.enter_context(tc.tile_pool(name="mask", bufs=2))
        mnp = ctx.enter_context(tc.tile_pool(name="maskneg", bufs=2))
        ap = ctx.enter_context(tc.tile_pool(name="attn", bufs=2))
        atp = ctx.enter_context(tc.tile_pool(name="attnT", bufs=4))
        stp = ctx.enter_context(tc.tile_pool(name="stats", bufs=4))
        mxp = ctx.enter_context(tc.tile_pool(name="mix", bufs=2))
        mtp = ctx.enter_context(tc.tile_pool(name="mixT", bufs=2))
        osb = ctx.enter_context(tc.tile_pool(name="osb", bufs=2))
        sc_ps = ctx.enter_context(tc.tile_pool(name="sc_ps", bufs=4, space="PSUM"))
        tr_ps = ctx.enter_context(tc.tile_pool(name="tr_ps", bufs=2, space="PSUM"))
        mix_ps = ctx.enter_context(tc.tile_pool(name="mix_ps", bufs=1, space="PSUM"))
        pr_ps = ctx.enter_context(tc.tile_pool(name="pr_ps", bufs=1, space="PSUM"))

        for qt in range(QT):
            qsl = slice(qt * P, (qt + 1) * P)
            mk = mq.tile([P, K], U8)
            nc.sync.dma_start(out=mk[:], in_=x_mask[qsl, :])
            mneg = mnp.tile([P, K], BF16)
            nc.gpsimd.tensor_scalar_mul(mneg[:], mk[:], NEGBIG)

            att = ap.tile([P, K], F32)
            stats = stp.tile([P, 16], F32)

            # scores: 4 quarters x 2 chunks; PSUM accumulates over h
            for quarter in range(4):
                pss = [
                    sc_ps.tile([P, 512], F32, name=f"scps{quarter}_{i2}", tag="scps")
                    for i2 in range(2)
                ]
                steps = (
                    [(outTh[h], ctxTh[h]) for h in range(HT)]
                    + [(outTl[h], ctxTh[h]) for h in range(HT)]
                    + [(outTh[h], ctxTl[h]) for h in range(HT)]
                )
                for si, (lt, rt) in enumerate(steps):
                    for i in range(2):
                        n = quarter * 2 + i
                        nc.tensor.matmul(
                            pss[i][:],
                            lhsT=lt[:, qsl],
                            rhs=rt[:, n * 512 : (n + 1) * 512],
                            start=(si == 0),
                            stop=(si == len(steps) - 1),
                        )
                for i in range(2):
                    n = quarter * 2 + i
                    nc.vector.tensor_tensor(
                        out=att[:, n * 512 : (n + 1) * 512],
                        in0=pss[i][:],
                        in1=mneg[:, n * 512 : (n + 1) * 512],
                        op=AL.add,
                    )
                    nc.vector.tensor_reduce(
                        out=stats[:, n : n + 1],
                        in_=att[:, n * 512 : (n + 1) * 512],
                        axis=AX.X,
                        op=AL.max,
                    )

            # softmax over the full row
            nc.vector.tensor_reduce(
                out=stats[:, 8:9], in_=stats[:, 0:NCH], axis=AX.X, op=AL.max,
                negate=True,
            )
            nc.scalar.activation(
                out=att[:], in_=att[:], func=AF.Exp,
                bias=stats[:, 8:9], scale=1.0, accum_out=stats[:, 9:10],
            )
            nc.vector.reciprocal(stats[:, 10:11], stats[:, 9:10])
            nc.gpsimd.tensor_scalar_mul(att[:], att[:], stats[:, 10:11])

            nc.sync.dma_start(out=y_attn[qsl, :], in_=att[:])

            # attn^T (bf16) + mix = attn @ context, software-pipelined on PE
            mxps = mix_ps.tile([P, 512], F32)
            prev = None
            for kb in range(KT // 4):
                tp = tr_ps.tile([P, 512], F32)
                for j in range(4):
                    nc.tensor.matmul(
                        tp[:, j * P : (j + 1) * P],
                        lhsT=att[:, (kb * 4 + j) * P : (kb * 4 + j + 1) * P],
                        rhs=ident[:],
                        is_transpose=True,
                        start=True,
                        stop=True,
                    )
                at_sb = atp.tile([P, 512], F16)
                nc.scalar.copy(at_sb[:], tp[:])
                if prev is not None:
                    pkb, pat = prev
                    for j in range(4):
                        kt = pkb * 4 + j
                        nc.tensor.matmul(
                            mxps[:],
                            lhsT=pat[:, j * P : (j + 1) * P],
                            rhs=cbf[kt][:],
                            start=(kt == 0),
                            stop=False,
                        )
                prev = (kb, at_sb)
            pkb, pat = prev
            for j in range(4):
                kt = pkb * 4 + j
                nc.tensor.matmul(
                    mxps[:],
                    lhsT=pat[:, j * P : (j + 1) * P],
                    rhs=cbf[kt][:],
                    start=False,
                    stop=(kt == KT - 1),
                )
            mx = mxp.tile([P, H], F32)
            nc.scalar.copy(mx[:], mxps[:])

            # mix^T
            mtps = tr_ps.tile([P, 512], F32, name="mtps", tag="tp")
            for j in range(HT):
                nc.tensor.matmul(
                    mtps[:, j * P : (j + 1) * P],
                    lhsT=mx[:, j * P : (j + 1) * P],
                    rhs=ident[:],
                    is_transpose=True,
                    start=True,
                    stop=True,
                )
            mt = mtp.tile([P, H], F16)
            nc.vector.tensor_copy(mt[:], mtps[:])

            # proj: out = tanh([mix, output] @ W_out^T + b_out)
            prps = pr_ps.tile([P, H], F32)
            for c in range(2 * H // P):
                if c < HT:
                    lhsT = mt[:, c * P : (c + 1) * P]
                else:
                    lhsT = outTh[c - HT][:, qsl]
                nc.tensor.matmul(
                    prps[:],
                    lhsT=lhsT,
                    rhs=wT[c][:],
                    start=(c == 0),
                    stop=False,
                )
            nc.tensor.matmul(
                prps[:],
                lhsT=ones[:, :],
                rhs=bias_sb[:],
                start=False,
                stop=True,
            )

            ot = osb.tile([P, H], F32)
            nc.scalar.activation(out=ot[:], in_=prps[:], func=AF.Tanh)
            nc.sync.dma_start(out=y_out[qsl, :], in_=ot[:])

    return nc


_NC_CACHE = None


def _get_nc():
    global _NC_CACHE
    if _NC_CACHE is None:
        _NC_CACHE = build()
    return _NC_CACHE


def kernel(output, context, mask, W_out, b_out):
    from concourse.bass_utils import run_bass_kernel_spmd

    _setup_jax_cache()

    output = np.ascontiguousarray(np.asarray(output), dtype=np.float32)
    context = np.ascontiguousarray(np.asarray(context), dtype=np.float32)
    mask_u8 = np.ascontiguousarray(np.asarray(mask)).astype(np.uint8)
    W = np.ascontiguousarray(np.asarray(W_out), dtype=np.float32)
    b = np.ascontiguousarray(np.asarray(b_out), dtype=np.float32).reshape(1, H)

    nc = _get_nc()
    _split_multi_waits(nc)
    in_maps = [
        {
            "x_output": output[i],
            "x_context": context[i],
            "x_mask": mask_u8[i],
            "x_wout": W,
            "x_bout": b,
        }
        for i in range(B)
    ]
    res = run_bass_kernel_spmd(nc, in_maps, list(range(B)))
    out = np.stack([res.results[i]["y_out"] for i in range(B)])
    attn = np.stack([res.results[i]["y_attn"] for i in range(B)])
    return out, attn
